# revision 27
# baseline (speedup 1.0000x reference)
"""Segment-mean pooling (segment_sum / counts) + Linear, on 8 TRN2 NeuronCores.

Strategy: segment-ownership sharding.  The host sorts rows by dst_idx and
routes each row to the core that owns its segment range (core i owns
segments [512*i, 512*(i+1))), so no collectives are needed; the host
concatenates the 8 output shards.

Per core the 512 segments split into 4 tiles of 128 segments.  All
accumulation matmuls are full-width M=128 (stationary is a [128, 128]
one-hot), which keeps the PE HAM activity monitor fed so the clock stays
at 2.4 GHz, and every 128-row chunk of x costs exactly one N=256 matmul:

  Band pass: the host packs the first C=16 rows of every segment into a
  dense band (fill ~99%); chunk c covers segs [8c, 8c+8) and its
  stationary is one of 16 fixed patterns (built on-device by DVE
  is_equal against an iota row).

  Tail pass: rows with rank >= 16 are packed densely in segment order,
  split at 128-segment tile boundaries so each tail chunk maps into one
  PSUM tile.  Each chunk ships a [128] relative-segment-index vector;
  DVE builds its [128, 128] one-hot, and one matmul accumulates it.
  Chunk counts per tile are maxed across cores (SPMD graph identity);
  short cores pad with zero rows / relidx=999 (one-hot of zeros).

Throughput notes (from baseline trace analysis): each dma_start costs
~650ns of issue time on its HWDGE ring, so data ships as ~11 large
transfers; one semaphore per DMA (shared-sem
intermediate thresholds are unsound: per-engine completions of
different transfers interleave); the PE is warmed with ~7us of junk
matmuls at t=0, bridging until the first data lands, so the band pass
runs at 2.4 GHz, not 1.2.

Epilogue per tile (pipelined under the DMA stream of later tiles):
fence, DVE-copy pooled sums to SBUF f16, PE-transpose to [h, s], apply
the Linear as out[s, j] = sums_T[:, s].T @ W.T[h, j], then one fused DVE
op scales rows by 1/(count+eps) (host bincount reciprocal) and adds the
bias, and sync-ring DMAs the [128, 256] f32 tile out.  Only the last
tile's epilogue is exposed past the end of the input stream.
"""

import numpy as np

import concourse.bass as bass
import concourse.mybir as mybir
from concourse.bass_utils import run_bass_kernel_spmd

N_CORES = 8
S_TOTAL = 4096
S_PER = S_TOTAL // N_CORES  # 512 segments per core
H = 256
EPS = np.float32(1e-8)
C = 16  # band capacity (rows per segment); must divide 128
KB = S_PER * C // 128  # 64 band chunks
NTILE = 4  # 128-seg tiles per core
PAD_IDX = 999.0  # relidx sentinel; never matches iota [0, 128)

_graph_cache: dict = {}


def _build(NT: tuple) -> "bass.Bass":
    """NT[t] = tail chunks for tile t (shared across cores)."""
    f16 = mybir.dt.float16
    f32 = mybir.dt.float32
    NTsum = sum(NT)
    iob = 264 + NTsum + (-(264 + NTsum) % 8)  # 32B-aligned iota start
    ncol_f = iob + 250  # invc8 | bb | relidx | pad | iota248 | patscA | patscB

    nc = bass.Bass()

    xb_d = nc.declare_dram_parameter("xb", [128, KB, H], f16, isOutput=False)
    xt_d = nc.declare_dram_parameter("xt", [128, NTsum, H], f16, isOutput=False)
    ch_d = nc.declare_dram_parameter("ch", [128, 896], f16, isOutput=False)
    cf_d = nc.declare_dram_parameter("cf", [128, ncol_f], f32, isOutput=False)
    out_d = nc.declare_dram_parameter("out", [S_PER, H], f32, isOutput=True)

    from contextlib import ExitStack

    with ExitStack() as ctx:
        xbb = ctx.enter_context(nc.sbuf_tensor("xbb", [128, KB, H], f16))
        xtt = ctx.enter_context(nc.sbuf_tensor("xtt", [128, NTsum, H], f16))
        ch = ctx.enter_context(nc.sbuf_tensor("ch_sb", [128, 896], f16))
        cf = ctx.enter_context(nc.sbuf_tensor("cf_sb", [128, ncol_f], f32))
        pat = ctx.enter_context(nc.sbuf_tensor("pat", [128, 248], f16))
        patB = ctx.enter_context(nc.sbuf_tensor("patB", [128, 240], f16))
        oh = ctx.enter_context(nc.sbuf_tensor("oh", [128, NTsum, 128], f16))
        pool = ctx.enter_context(nc.sbuf_tensor("pool", [128, NTILE, H], f16))
        sums2 = ctx.enter_context(nc.sbuf_tensor("sums2", [128, 2, 128], f16))
        outb = ctx.enter_context(nc.sbuf_tensor("outb", [128, NTILE, H], f32))
        # every PSUM tensor is one full private 2 KiB bank
        ps_s = [
            ctx.enter_context(nc.psum_tensor(f"ps_s{t}", [128, 512], f32))
            for t in range(NTILE)
        ]
        ps_t = [
            ctx.enter_context(nc.psum_tensor(f"ps_t{hb}", [128, 1024], f16))
            for hb in range(2)
        ]
        ps_x = ctx.enter_context(nc.psum_tensor("ps_x", [128, 512], f32))
        scf = ctx.enter_context(nc.semaphore("scf"))
        sch = ctx.enter_context(nc.semaphore("sch"))
        sb = [ctx.enter_context(nc.semaphore(f"sb{t}")) for t in range(4)]
        sb0b = ctx.enter_context(nc.semaphore("sb0b"))
        st = [ctx.enter_context(nc.semaphore(f"st{t}")) for t in range(4)]
        st3b = ctx.enter_context(nc.semaphore("st3b"))
        s_pe = ctx.enter_context(nc.semaphore("s_pe"))
        s_dv = ctx.enter_context(nc.semaphore("s_dv"))
        s_od = ctx.enter_context(nc.semaphore("s_od"))

        block = ctx.enter_context(nc.Block())

        zlhs = ch[0:1, 0:128]  # junk 1-partition stationary for fences
        zrhs = ch[0:1, 0:256]  # junk rhs; ps_x is never read

        # s_dv value map (DVE producer)
        d_pat = 2
        d_oh = [2 + sum(NT[:t + 1]) for t in range(NTILE)]
        d_cp = [2 + NTsum + 3 * t + 1 for t in range(NTILE)]
        d_cp2 = [2 + NTsum + 3 * t + 2 for t in range(NTILE)]
        d_oe = [2 + NTsum + 3 * t + 3 for t in range(NTILE)]
        # s_pe value map (PE producer): fence_t, tr_t, lin_t
        p_fence = [1, 3, 6, 9]
        p_tr = [2, 5, 8, 11]
        p_lin = [4, 7, 10, 12]
        nt3a = NT[3] - 2 if NT[3] >= 3 else NT[3]
        toff = [sum(NT[:t]) for t in range(NTILE)]

        @block.scalar
        def _(scalar):
            # all input DMAs, one ring (q10), in consumption order
            # one semaphore per DMA: per-engine completions of different
            # transfers on one queue interleave, so an intermediate
            # threshold on a shared sem does NOT imply the earlier
            # transfer fully landed
            scalar.dma_start(out=xbb[:, 0:4, :], in_=xb_d[:, 0:4, :]).then_inc(
                sb[0], 16
            )
            scalar.dma_start(out=xbb[:, 4:16, :], in_=xb_d[:, 4:16, :]).then_inc(
                sb0b, 16
            )
            for t in range(NTILE):
                if t > 0:
                    scalar.dma_start(
                        out=xbb[:, 16 * t : 16 * (t + 1), :],
                        in_=xb_d[:, 16 * t : 16 * (t + 1), :],
                    ).then_inc(sb[t], 16)
                if t < 3 or nt3a == NT[3]:
                    scalar.dma_start(
                        out=xtt[:, toff[t] : toff[t] + NT[t], :],
                        in_=xt_d[:, toff[t] : toff[t] + NT[t], :],
                    ).then_inc(st[t], 16)
                else:
                    scalar.dma_start(
                        out=xtt[:, toff[3] : toff[3] + nt3a, :],
                        in_=xt_d[:, toff[3] : toff[3] + nt3a, :],
                    ).then_inc(st[3], 16)
                    scalar.dma_start(
                        out=xtt[:, toff[3] + nt3a : toff[3] + NT[3], :],
                        in_=xt_d[:, toff[3] + nt3a : toff[3] + NT[3], :],
                    ).then_inc(st3b, 16)

        @block.vector
        def _(vector):
            vector.wait_ge(scf, 16)
            io = iob  # iota248 start in cf (32B-aligned)
            vector.tensor_scalar(
                out=pat[:, :],
                in0=cf[:, io : io + 248],
                scalar1=cf[:, io + 248 : io + 249],
                scalar2=None,
                op0=mybir.AluOpType.is_equal,
            ).then_inc(s_dv, 1)
            vector.tensor_scalar(
                out=patB[:, :],
                in0=cf[:, io : io + 240],
                scalar1=cf[:, io + 249 : io + 250],
                scalar2=None,
                op0=mybir.AluOpType.is_equal,
            ).then_inc(s_dv, 1)
            for k in range(NTsum):
                vector.tensor_scalar(
                    out=oh[:, k, :],
                    in0=cf[:, io : io + 128],
                    scalar1=cf[:, 264 + k : 265 + k],
                    scalar2=None,
                    op0=mybir.AluOpType.is_equal,
                ).then_inc(s_dv, 1)
            for t in range(NTILE):
                vector.wait_ge(s_pe, p_fence[t])
                vector.tensor_copy(out=pool[:, t, :], in_=ps_s[t][:, 0:H]).then_inc(
                    s_dv, 1
                )
                vector.wait_ge(s_pe, p_tr[t])
                vector.tensor_copy(out=sums2[:, 0, :], in_=ps_t[0][:, 0:128])
                vector.tensor_copy(
                    out=sums2[:, 1, :], in_=ps_t[1][:, 0:128]
                ).then_inc(s_dv, 1)
                vector.wait_ge(s_pe, p_lin[t])
                vector.scalar_tensor_tensor(
                    out=outb[:, t, :],
                    in0=ps_s[t][:, 0:H],
                    scalar=cf[:, t : t + 1],
                    in1=cf[:, 8 : 8 + H],
                    op0=mybir.AluOpType.mult,
                    op1=mybir.AluOpType.add,
                ).then_inc(s_dv, 1)

        @block.tensor
        def _(tensor):
            def fence(inc=True):
                ins = tensor.matmul(
                    ps_x[:, 0:256], zlhs, zrhs, start=True, stop=True,
                    skip_group_check=True,
                )
                if inc:
                    ins.then_inc(s_pe, 1)

            def band(t):
                tensor.wait_ge(sb[t], 16)
                for i in range(16):
                    if t == 0 and i == 4:
                        tensor.wait_ge(sb0b, 16)
                    tensor.matmul(
                        ps_s[t][:, 0:H],
                        pat[:, 120 - 8 * i : 248 - 8 * i]
                        if i % 2 == 1
                        else patB[:, 112 - 8 * i : 240 - 8 * i],
                        xbb[:, 16 * t + i, :],
                        start=(i == 0),
                        stop=(i == 15 and NT[t] == 0),
                        skip_group_check=True,
                    )

            def tail(t):
                if t < 3 or nt3a == NT[3]:
                    tensor.wait_ge(st[t], 16)
                    parts = [range(NT[t])]
                else:
                    parts = [range(nt3a), range(nt3a, NT[3])]
                    tensor.wait_ge(st[3], 16)
                tensor.wait_ge(s_dv, d_oh[t])
                for pi, rng in enumerate(parts):
                    if pi == 1:
                        tensor.wait_ge(st3b, 16)
                    for k in rng:
                        tensor.matmul(
                            ps_s[t][:, 0:H],
                            oh[:, toff[t] + k, :],
                            xtt[:, toff[t] + k, :],
                            start=False,
                            stop=(k == NT[t] - 1),
                            skip_group_check=True,
                        )

            def trans(t):
                if t == 0:
                    tensor.wait_ge(sch, 16)
                tensor.wait_ge(s_dv, d_cp[t])
                for hb in range(2):
                    tensor.transpose(
                        ps_t[hb][:, 0:128],
                        pool[:, t, 128 * hb : 128 * (hb + 1)],
                        ch[:, 256:384],
                    )
                fence()  # drain guard before DVE reads ps_t -> inc s_pe

            def linear(t):
                tensor.wait_ge(s_dv, d_cp2[t])
                tensor.matmul(
                    ps_s[t][:, 0:H], sums2[:, 0, :], ch[:, 384:640],
                    start=True, stop=False, skip_group_check=True,
                )
                tensor.matmul(
                    ps_s[t][:, 0:H], sums2[:, 1, :], ch[:, 640:896],
                    start=False, stop=True, skip_group_check=True,
                )
                fence()

            # ~5us of sustained full-width junk matmuls, no data deps:
            # HAM unthrottles the PE clock to 2.4 GHz before real work
            import os as _os

            if _os.environ.get("KSIM"):  # sim rejects uninitialized reads
                tensor.wait_ge(sch, 16)
            warm_st = ch[:, 0:128] if _os.environ.get("KSIM") else xbb[:, 0, 0:128]
            warm_mv = ch[:, 0:512] if _os.environ.get("KSIM") else xbb[:, 0:2, :]
            for _ in range(14):
                tensor.matmul(
                    ps_x[:, 0:512], warm_st, warm_mv,
                    start=True, stop=True, skip_group_check=True,
                )
            tensor.wait_ge(s_dv, d_pat)
            band(0)
            tail(0)
            fence()
            band(1)
            trans(0)
            tail(1)
            fence()
            linear(0)
            band(2)
            trans(1)
            tail(2)
            fence()
            linear(1)
            band(3)
            trans(2)
            tail(3)
            fence()
            linear(2)
            trans(3)
            linear(3)

        @block.sync
        def _(sync):
            # consts ride the sync ring's queue in parallel with the data
            # stream; cf is needed first (tail one-hots), ch at transpose 0
            sync.dma_start(out=cf[:, :], in_=cf_d[:, :]).then_inc(scf, 16)
            sync.dma_start(out=ch[:, :], in_=ch_d[:, :]).then_inc(sch, 16)
            for t in range(NTILE):
                sync.wait_ge(s_dv, d_oe[t])
                sync.dma_start(
                    out=out_d[128 * t : 128 * (t + 1), :], in_=outb[:, t, :]
                ).then_inc(s_od, 16)
            sync.wait_ge(s_od, 64)

    return nc


def kernel(x, dst_idx, dst_size, W, b):
    x = np.asarray(x)
    idx = np.asarray(dst_idx).astype(np.int64)
    W = np.asarray(W, dtype=np.float32)
    b = np.asarray(b, dtype=np.float32)
    S = int(dst_size)
    assert S == S_TOTAL and x.shape[1] == H

    counts = np.bincount(idx, minlength=S).astype(np.float32)
    inv = np.float32(1.0) / (counts + EPS)  # [4096] f32

    order = np.argsort(idx, kind="stable")
    sidx = idx[order]
    bounds = np.searchsorted(sidx, np.arange(0, S + 1, S_PER))

    x16 = x.astype(np.float16)

    # split each core's rows into band (rank < C) and tail (rank >= C)
    bands, tails, tsegs = [], [], []
    for i in range(N_CORES):
        lo_i, hi_i = bounds[i], bounds[i + 1]
        n_i = hi_i - lo_i
        li = (sidx[lo_i:hi_i] - S_PER * i).astype(np.int64)
        rows = order[lo_i:hi_i]
        starts = np.searchsorted(li, np.arange(S_PER + 1))
        rank = np.arange(n_i) - starts[li]
        bm = rank < C
        sa = li[bm]
        xband = np.zeros((128, KB, H), dtype=np.float16)
        xband[(sa % 8) * 16 + rank[bm], sa // 8] = x16[rows[bm]]
        bands.append(xband)
        tm = ~bm
        tails.append(x16[rows[tm]])
        tsegs.append(li[tm])

    # shared per-tile tail chunk counts (graph identity across cores)
    NT = []
    for t in range(NTILE):
        m = max(
            int(((s >= 128 * t) & (s < 128 * (t + 1))).sum()) for s in tsegs
        )
        NT.append(-(-m // 128))
    NT = tuple(NT)
    NTsum = sum(NT)
    toff = [sum(NT[:t]) for t in range(NTILE)]

    key = NT
    nc = _graph_cache.get(key)
    if nc is None:
        nc = _build(NT)
        _graph_cache[key] = nc

    # shared f16 consts: iota248 | ident | W.T packed per h-half
    ch_np = np.zeros((128, 896), dtype=np.float16)
    ch_np[:, 0:248] = np.arange(248, dtype=np.float16)
    ch_np[:, 256:384] = np.eye(128, dtype=np.float16)
    WT = np.ascontiguousarray(W.T).astype(np.float16)  # [h, j]
    ch_np[:, 384:640] = WT[0:128, :]
    ch_np[:, 640:896] = WT[128:256, :]
    bbt = np.tile(b, (128, 1)).astype(np.float32)

    in_maps = []
    for i in range(N_CORES):
        xt_np = np.zeros((128, NTsum, H), dtype=np.float16)
        relidx = np.full((128, NTsum), PAD_IDX, dtype=np.float32)
        s_i, x_i = tsegs[i], tails[i]
        for t in range(NTILE):
            m = (s_i >= 128 * t) & (s_i < 128 * (t + 1))
            st, xt_rows = s_i[m], x_i[m]
            r = np.arange(len(st))
            xt_np[r % 128, toff[t] + r // 128] = xt_rows
            relidx[r % 128, toff[t] + r // 128] = st - 128 * t
        iob = 264 + NTsum + (-(264 + NTsum) % 8)
        cf_np = np.zeros((128, iob + 250), dtype=np.float32)
        cf_np[:, 0:4] = inv[S_PER * i : S_PER * (i + 1)].reshape(4, 128).T
        cf_np[:, 8 : 8 + H] = bbt
        cf_np[:, 264 : 264 + NTsum] = relidx
        cf_np[:, iob : iob + 248] = np.arange(248)
        cf_np[:, iob + 248] = np.arange(128) // C + 120
        cf_np[:, iob + 249] = np.arange(128) // C + 112
        in_maps.append(
            {"xb": bands[i], "xt": xt_np, "ch": ch_np, "cf": cf_np}
        )

    res = run_bass_kernel_spmd(nc, in_maps, core_ids=list(range(N_CORES)))
    return np.concatenate([res.results[i]["out"] for i in range(N_CORES)], axis=0)


# revision 29
# speedup vs baseline: 1.0174x; 1.0174x over previous
"""Segment-mean pooling (segment_sum / counts) + Linear, on 8 TRN2 NeuronCores.

Strategy: segment-ownership sharding.  The host sorts rows by dst_idx and
routes each row to the core that owns its segment range (core i owns
segments [512*i, 512*(i+1))), so no collectives are needed; the host
concatenates the 8 output shards.

Per core the 512 segments split into 4 tiles of 128 segments.  All
accumulation matmuls are full-width M=128 (stationary is a [128, 128]
one-hot), which keeps the PE HAM activity monitor fed so the clock stays
at 2.4 GHz, and every 128-row chunk of x costs exactly one N=256 matmul:

  Band pass: the host packs the first C=16 rows of every segment into a
  dense band (fill ~99%); chunk c covers segs [8c, 8c+8) and its
  stationary is one of 16 fixed patterns (built on-device by DVE
  is_equal against an iota row).

  Tail pass: rows with rank >= 16 are packed densely in segment order,
  split at 128-segment tile boundaries so each tail chunk maps into one
  PSUM tile.  Each chunk ships a [128] relative-segment-index vector;
  DVE builds its [128, 128] one-hot, and one matmul accumulates it.
  Chunk counts per tile are maxed across cores (SPMD graph identity);
  short cores pad with zero rows / relidx=999 (one-hot of zeros).

Throughput notes (from baseline trace analysis): each dma_start costs
~650ns of issue time on its HWDGE ring, so data ships as ~11 large
transfers; one semaphore per DMA (shared-sem
intermediate thresholds are unsound: per-engine completions of
different transfers interleave); the PE is warmed with ~7us of junk
matmuls at t=0, bridging until the first data lands, so the band pass
runs at 2.4 GHz, not 1.2.

Epilogue per tile (pipelined under the DMA stream of later tiles):
fence, DVE-copy pooled sums to SBUF f16, PE-transpose to [h, s], apply
the Linear as out[s, j] = sums_T[:, s].T @ W.T[h, j], then one fused DVE
op scales rows by 1/(count+eps) (host bincount reciprocal) and adds the
bias, and sync-ring DMAs the [128, 256] f32 tile out.  Only the last
tile's epilogue is exposed past the end of the input stream.
"""

import numpy as np

import concourse.bass as bass
import concourse.mybir as mybir
from concourse.bass_utils import run_bass_kernel_spmd

N_CORES = 8
S_TOTAL = 4096
S_PER = S_TOTAL // N_CORES  # 512 segments per core
H = 256
EPS = np.float32(1e-8)
C = 16  # band capacity (rows per segment); must divide 128
KB = S_PER * C // 128  # 64 band chunks
NTILE = 4  # 128-seg tiles per core
PAD_IDX = 999.0  # relidx sentinel; never matches iota [0, 128)

_graph_cache: dict = {}


def _build(NT: tuple) -> "bass.Bass":
    """NT[t] = tail chunks for tile t (shared across cores)."""
    f16 = mybir.dt.float16
    f32 = mybir.dt.float32
    NTsum = sum(NT)
    iob = 264 + NTsum + (-(264 + NTsum) % 8)  # 32B-aligned iota start
    ncol_f = iob + 250  # invc8 | bb | relidx | pad | iota248 | patscA | patscB

    nc = bass.Bass()

    xb_d = nc.declare_dram_parameter("xb", [128, KB, H], f16, isOutput=False)
    xt_d = nc.declare_dram_parameter("xt", [128, NTsum, H], f16, isOutput=False)
    ch_d = nc.declare_dram_parameter("ch", [128, 896], f16, isOutput=False)
    cf_d = nc.declare_dram_parameter("cf", [128, ncol_f], f32, isOutput=False)
    out_d = nc.declare_dram_parameter("out", [S_PER, H], f32, isOutput=True)

    from contextlib import ExitStack

    with ExitStack() as ctx:
        xbb = ctx.enter_context(nc.sbuf_tensor("xbb", [128, KB, H], f16))
        xtt = ctx.enter_context(nc.sbuf_tensor("xtt", [128, NTsum, H], f16))
        ch = ctx.enter_context(nc.sbuf_tensor("ch_sb", [128, 896], f16))
        cf = ctx.enter_context(nc.sbuf_tensor("cf_sb", [128, ncol_f], f32))
        pat = ctx.enter_context(nc.sbuf_tensor("pat", [128, 248], f16))
        patB = ctx.enter_context(nc.sbuf_tensor("patB", [128, 240], f16))
        oh = ctx.enter_context(nc.sbuf_tensor("oh", [128, NTsum, 128], f16))
        pool = ctx.enter_context(nc.sbuf_tensor("pool", [128, NTILE, H], f16))
        sums2 = ctx.enter_context(nc.sbuf_tensor("sums2", [128, 2, 128], f16))
        outb = ctx.enter_context(nc.sbuf_tensor("outb", [128, NTILE, H], f32))
        # every PSUM tensor is one full private 2 KiB bank
        ps_s = [
            ctx.enter_context(nc.psum_tensor(f"ps_s{t}", [128, 512], f32))
            for t in range(NTILE)
        ]
        ps_t = [
            ctx.enter_context(nc.psum_tensor(f"ps_t{hb}", [128, 1024], f16))
            for hb in range(2)
        ]
        ps_x = ctx.enter_context(nc.psum_tensor("ps_x", [128, 512], f32))
        scf = ctx.enter_context(nc.semaphore("scf"))
        sch = ctx.enter_context(nc.semaphore("sch"))
        sb = [ctx.enter_context(nc.semaphore(f"sb{t}")) for t in range(4)]
        sb0b = ctx.enter_context(nc.semaphore("sb0b"))
        st = [ctx.enter_context(nc.semaphore(f"st{t}")) for t in range(4)]
        st3b = ctx.enter_context(nc.semaphore("st3b"))
        s_pe = ctx.enter_context(nc.semaphore("s_pe"))
        s_dv = ctx.enter_context(nc.semaphore("s_dv"))
        s_od = ctx.enter_context(nc.semaphore("s_od"))

        block = ctx.enter_context(nc.Block())

        zlhs = ch[0:1, 0:128]  # junk 1-partition stationary for fences
        zrhs = ch[0:1, 0:256]  # junk rhs; ps_x is never read

        # s_dv value map (DVE producer)
        d_pat = 2
        d_oh = [2 + sum(NT[:t + 1]) for t in range(NTILE)]
        d_cp = [2 + NTsum + 3 * t + 1 for t in range(NTILE)]
        d_cp2 = [2 + NTsum + 3 * t + 2 for t in range(NTILE)]
        d_oe = [2 + NTsum + 3 * t + 3 for t in range(NTILE)]
        # s_pe value map (PE producer): fence_t, tr_t, lin_t
        p_fence = [1, 3, 6, 9]
        p_tr = [2, 5, 8, 11]
        p_lin = [4, 7, 10, 12]
        nt3a = NT[3] - 2 if NT[3] >= 3 else NT[3]
        toff = [sum(NT[:t]) for t in range(NTILE)]

        @block.scalar
        def _(scalar):
            # all input DMAs, one ring (q10), in consumption order
            # one semaphore per DMA: per-engine completions of different
            # transfers on one queue interleave, so an intermediate
            # threshold on a shared sem does NOT imply the earlier
            # transfer fully landed
            scalar.dma_start(out=xbb[:, 0:4, :], in_=xb_d[:, 0:4, :]).then_inc(
                sb[0], 16
            )
            scalar.dma_start(out=xbb[:, 4:16, :], in_=xb_d[:, 4:16, :]).then_inc(
                sb0b, 16
            )
            for t in range(NTILE):
                if t > 0:
                    scalar.dma_start(
                        out=xbb[:, 16 * t : 16 * (t + 1), :],
                        in_=xb_d[:, 16 * t : 16 * (t + 1), :],
                    ).then_inc(sb[t], 16)
                if t < 3 or nt3a == NT[3]:
                    scalar.dma_start(
                        out=xtt[:, toff[t] : toff[t] + NT[t], :],
                        in_=xt_d[:, toff[t] : toff[t] + NT[t], :],
                    ).then_inc(st[t], 16)
                else:
                    scalar.dma_start(
                        out=xtt[:, toff[3] : toff[3] + nt3a, :],
                        in_=xt_d[:, toff[3] : toff[3] + nt3a, :],
                    ).then_inc(st[3], 16)
                    scalar.dma_start(
                        out=xtt[:, toff[3] + nt3a : toff[3] + NT[3], :],
                        in_=xt_d[:, toff[3] + nt3a : toff[3] + NT[3], :],
                    ).then_inc(st3b, 16)

        @block.vector
        def _(vector):
            vector.wait_ge(scf, 16)
            io = iob  # iota248 start in cf (32B-aligned)
            vector.tensor_scalar(
                out=pat[:, :],
                in0=cf[:, io : io + 248],
                scalar1=cf[:, io + 248 : io + 249],
                scalar2=None,
                op0=mybir.AluOpType.is_equal,
            ).then_inc(s_dv, 1)
            vector.tensor_scalar(
                out=patB[:, :],
                in0=cf[:, io : io + 240],
                scalar1=cf[:, io + 249 : io + 250],
                scalar2=None,
                op0=mybir.AluOpType.is_equal,
            ).then_inc(s_dv, 1)
            for k in range(NTsum):
                vector.tensor_scalar(
                    out=oh[:, k, :],
                    in0=cf[:, io : io + 128],
                    scalar1=cf[:, 264 + k : 265 + k],
                    scalar2=None,
                    op0=mybir.AluOpType.is_equal,
                ).then_inc(s_dv, 1)
            for t in range(NTILE):
                vector.wait_ge(s_pe, p_fence[t])
                vector.tensor_copy(out=pool[:, t, :], in_=ps_s[t][:, 0:H]).then_inc(
                    s_dv, 1
                )
                vector.wait_ge(s_pe, p_tr[t])
                vector.tensor_copy(out=sums2[:, 0, :], in_=ps_t[0][:, 0:128])
                vector.tensor_copy(
                    out=sums2[:, 1, :], in_=ps_t[1][:, 0:128]
                ).then_inc(s_dv, 1)
                vector.wait_ge(s_pe, p_lin[t])
                vector.scalar_tensor_tensor(
                    out=outb[:, t, :],
                    in0=ps_s[t][:, 0:H],
                    scalar=cf[:, t : t + 1],
                    in1=cf[:, 8 : 8 + H],
                    op0=mybir.AluOpType.mult,
                    op1=mybir.AluOpType.add,
                ).then_inc(s_dv, 1)

        @block.tensor
        def _(tensor):
            def fence(inc=True):
                ins = tensor.matmul(
                    ps_x[:, 0:256], zlhs, zrhs, start=True, stop=True,
                    skip_group_check=True,
                )
                if inc:
                    ins.then_inc(s_pe, 1)

            def band(t):
                tensor.wait_ge(sb[t], 16)
                for i in range(16):
                    if t == 0 and i == 4:
                        tensor.wait_ge(sb0b, 16)
                    tensor.matmul(
                        ps_s[t][:, 0:H],
                        pat[:, 120 - 8 * i : 248 - 8 * i]
                        if i % 2 == 1
                        else patB[:, 112 - 8 * i : 240 - 8 * i],
                        xbb[:, 16 * t + i, :],
                        start=(i == 0),
                        stop=(i == 15 and NT[t] == 0),
                        skip_group_check=True,
                    )

            def tail(t):
                if t < 3 or nt3a == NT[3]:
                    tensor.wait_ge(st[t], 16)
                    parts = [range(NT[t])]
                else:
                    parts = [range(nt3a), range(nt3a, NT[3])]
                    tensor.wait_ge(st[3], 16)
                tensor.wait_ge(s_dv, d_oh[t])
                for pi, rng in enumerate(parts):
                    if pi == 1:
                        tensor.wait_ge(st3b, 16)
                    for k in rng:
                        tensor.matmul(
                            ps_s[t][:, 0:H],
                            oh[:, toff[t] + k, :],
                            xtt[:, toff[t] + k, :],
                            start=False,
                            stop=(k == NT[t] - 1),
                            skip_group_check=True,
                        )

            def trans(t):
                if t == 0:
                    tensor.wait_ge(sch, 16)
                tensor.wait_ge(s_dv, d_cp[t])
                for hb in range(2):
                    tensor.transpose(
                        ps_t[hb][:, 0:128],
                        pool[:, t, 128 * hb : 128 * (hb + 1)],
                        ch[:, 256:384],
                    )
                fence()  # drain guard before DVE reads ps_t -> inc s_pe

            def linear(t):
                tensor.wait_ge(s_dv, d_cp2[t])
                tensor.matmul(
                    ps_s[t][:, 0:H], sums2[:, 0, :], ch[:, 384:640],
                    start=True, stop=False, skip_group_check=True,
                )
                tensor.matmul(
                    ps_s[t][:, 0:H], sums2[:, 1, :], ch[:, 640:896],
                    start=False, stop=True, skip_group_check=True,
                )
                fence()

            # ~5us of sustained full-width junk matmuls, no data deps:
            # HAM unthrottles the PE clock to 2.4 GHz before real work
            import os as _os

            if _os.environ.get("KSIM"):  # sim rejects uninitialized reads
                tensor.wait_ge(sch, 16)
            warm_st = ch[:, 0:128] if _os.environ.get("KSIM") else xbb[:, 0, 0:128]
            warm_mv = ch[:, 0:512] if _os.environ.get("KSIM") else xbb[:, 0:2, :]
            for _ in range(14):
                tensor.matmul(
                    ps_x[:, 0:512], warm_st, warm_mv,
                    start=True, stop=True, skip_group_check=True,
                )
            tensor.wait_ge(s_dv, d_pat)
            band(0)
            tail(0)
            fence()
            band(1)
            trans(0)
            tail(1)
            fence()
            linear(0)
            band(2)
            trans(1)
            tail(2)
            fence()
            linear(1)
            band(3)
            trans(2)
            tail(3)
            fence()
            linear(2)
            trans(3)
            linear(3)

        @block.sync
        def _(sync):
            # consts ride the sync ring's queue in parallel with the data
            # stream; cf is needed first (tail one-hots), ch at transpose 0
            sync.dma_start(out=cf[:, :], in_=cf_d[:, :]).then_inc(scf, 16)
            sync.dma_start(out=ch[:, :], in_=ch_d[:, :]).then_inc(sch, 16)
            for t in range(NTILE):
                sync.wait_ge(s_dv, d_oe[t])
                sync.dma_start(
                    out=out_d[128 * t : 128 * (t + 1), :], in_=outb[:, t, :]
                ).then_inc(s_od, 16)
            sync.wait_ge(s_od, 64)

    return nc


def kernel(x, dst_idx, dst_size, W, b):
    x = np.asarray(x)
    idx = np.asarray(dst_idx).astype(np.int64)
    W = np.asarray(W, dtype=np.float32)
    b = np.asarray(b, dtype=np.float32)
    S = int(dst_size)
    assert S == S_TOTAL and x.shape[1] == H

    counts = np.bincount(idx, minlength=S).astype(np.float32)
    inv = np.float32(1.0) / (counts + EPS)  # [4096] f32

    order = np.argsort(idx, kind="stable")
    sidx = idx[order]
    bounds = np.searchsorted(sidx, np.arange(0, S + 1, S_PER))

    x16 = x.astype(np.float16)

    # split each core's rows into band (rank < C) and tail (rank >= C)
    bands, tails, tsegs = [], [], []
    for i in range(N_CORES):
        lo_i, hi_i = bounds[i], bounds[i + 1]
        n_i = hi_i - lo_i
        li = (sidx[lo_i:hi_i] - S_PER * i).astype(np.int64)
        rows = order[lo_i:hi_i]
        starts = np.searchsorted(li, np.arange(S_PER + 1))
        rank = np.arange(n_i) - starts[li]
        bm = rank < C
        sa = li[bm]
        xband = np.zeros((128, KB, H), dtype=np.float16)
        xband[(sa % 8) * 16 + rank[bm], sa // 8] = x16[rows[bm]]
        bands.append(xband)
        tm = ~bm
        tails.append(x16[rows[tm]])
        tsegs.append(li[tm])

    # shared per-tile tail chunk counts (graph identity across cores)
    NT = []
    for t in range(NTILE):
        m = max(
            int(((s >= 128 * t) & (s < 128 * (t + 1))).sum()) for s in tsegs
        )
        NT.append(-(-m // 128))
    NT = tuple(NT)
    NTsum = sum(NT)
    toff = [sum(NT[:t]) for t in range(NTILE)]

    key = NT
    nc = _graph_cache.get(key)
    if nc is None:
        nc = _build(NT)
        _graph_cache[key] = nc

    # shared f16 consts: iota248 | ident | W.T packed per h-half
    ch_np = np.zeros((128, 896), dtype=np.float16)
    ch_np[:, 0:248] = np.arange(248, dtype=np.float16)
    ch_np[:, 256:384] = np.eye(128, dtype=np.float16)
    WT = np.ascontiguousarray(W.T).astype(np.float16)  # [h, j]
    ch_np[:, 384:640] = WT[0:128, :]
    ch_np[:, 640:896] = WT[128:256, :]
    bbt = np.tile(b, (128, 1)).astype(np.float32)

    in_maps = []
    for i in range(N_CORES):
        xt_np = np.zeros((128, NTsum, H), dtype=np.float16)
        relidx = np.full((128, NTsum), PAD_IDX, dtype=np.float32)
        s_i, x_i = tsegs[i], tails[i]
        for t in range(NTILE):
            m = (s_i >= 128 * t) & (s_i < 128 * (t + 1))
            st, xt_rows = s_i[m], x_i[m]
            r = np.arange(len(st))
            xt_np[r % 128, toff[t] + r // 128] = xt_rows
            relidx[r % 128, toff[t] + r // 128] = st - 128 * t
        iob = 264 + NTsum + (-(264 + NTsum) % 8)
        cf_np = np.zeros((128, iob + 250), dtype=np.float32)
        cf_np[:, 0:4] = inv[S_PER * i : S_PER * (i + 1)].reshape(4, 128).T
        cf_np[:, 8 : 8 + H] = bbt
        cf_np[:, 264 : 264 + NTsum] = relidx
        cf_np[:, iob : iob + 248] = np.arange(248)
        cf_np[:, iob + 248] = np.arange(128) // C + 120
        cf_np[:, iob + 249] = np.arange(128) // C + 112
        in_maps.append(
            {"xb": bands[i], "xt": xt_np, "ch": ch_np, "cf": cf_np}
        )

    res = run_bass_kernel_spmd(nc, in_maps, core_ids=list(range(N_CORES)))
    return np.concatenate([res.results[i]["out"] for i in range(N_CORES)], axis=0)
